# revision 14
# baseline (speedup 1.0000x reference)
"""DRQConv2d (dual-region quantized conv) Trainium2 kernel — v2.

Reference semantics:
  mask  = upsample8(avgpool8(x) >= 0.05)             per (b, c)
  xh    = where(mask, x, 1e-5);  xl = where(mask, 1e-5, x)
  qh    = clip(round(xh/sh), 0, 255) * sh            (uint8 fake-quant)
  ql    = clip(round(xl/sl), 0, 15) * sl             (uint4 fake-quant)
  qwh   = per-oc quant of w_high to +-127,  qwl = per-oc quant of w_low to +-7
  y     = conv3x3(qh, qwh) + conv3x3(ql, qwl)        (pad 1)

Key ideas vs the v1 baseline (151us):
  * Low conv runs in fp8 e4m3 with MatmulPerfMode.DoubleRow: quantized low
    activations (ints 0..15) and weights (ints +-7) are exactly representable
    in e4m3, and DoubleRow packs TWO 3x3 taps (2x128 contraction rows) into
    one PE instruction -> 9 taps in 5 matmuls instead of 9.  The per-oc scale
    ratio sv_h/sv_l is folded into the HIGH (bf16) weights instead of the low
    ones so the low weights stay exact integers; both convs share PSUM banks
    and one final evacuation scale by sv_l (adds ~0.1% error, gate is 2%).
  * The region mask is applied as a CLAMP BOUND instead of a multiply:
    v = x/s + MAGIC (ACT, scale+round fused); u = min(v, MAGIC + qmax*mask)
    (GpSimd, mask expanded only to [P, 7*56] block-row resolution and
    broadcast-viewed); q = max(u, MAGIC) - MAGIC (DVE tensor_scalar, 2x mode).
    Masked-out pixels clamp to exactly MAGIC -> quantize to 0, which matches
    the reference (1e-5 rounds to 0).  This removes the full-res mask
    expansion and the 1x-mode scalar_tensor_tensor of v1.
  * PSUM is laid out as two bank-aligned supertiles (chunks at 512-elem
    stride) so each image needs only 2 strided ACT evacuations + 1 DMA out.

Sharding: data-parallel over batch. 32 images -> 4 per core on 8 cores,
weights replicated; outputs concatenated on host. No collectives.
"""

import numpy as np

P = 128            # channels (both in and out) == partitions
B_TOTAL = 32
N_CORES = 8
BPC = B_TOTAL // N_CORES   # images per core
H = W = 56
HP = WP = H + 2    # zero-padded layout
NPIX = H * W       # 3136
NPAD = HP * WP     # 3364
NTAPS = 9
ROWS = 8
NCHUNK = H // ROWS                    # 7
NFREE = ROWS * W                      # 448 columns per matmul
BANK = 512                            # PSUM bank stride in f32 elements
MAGIC = float(np.float32(1.5 * 2 ** 23))   # fp32 round-to-nearest magic
POOL_K = 8
THRESH = 0.05


def build_program(nc, tc, aps, inv_sh, inv_sl, c_svh, c_svl, bpc=BPC):
    import bass_rust as _br
    import concourse.mybir as mybir
    from concourse.alu_op_type import AluOpType as op
    from concourse.masks import make_identity

    f32 = mybir.dt.float32
    bf16 = mybir.dt.bfloat16
    fp8 = mybir.dt.float8e4
    X = mybir.AxisListType.X
    DR = mybir.MatmulPerfMode.DoubleRow
    IDENT = mybir.ActivationFunctionType.Identity

    x_d, wh_d, wl_d, y_d = aps["x"], aps["w_high"], aps["w_low"], aps["y"]
    sum_thresh = float(np.float32(THRESH) * POOL_K * POOL_K)  # exact pow2 scale

    with (
        tc.tile_pool(name="consts", bufs=1) as consts,
        tc.tile_pool(name="wtmp", bufs=1) as wtmp,
        tc.tile_pool(name="psum", bufs=1, space="PSUM") as psum_pool,
        tc.tile_pool(name="acts", bufs=2) as acts,
        tc.tile_pool(name="qtiles", bufs=3) as qtiles,
        tc.tile_pool(name="outs", bufs=2) as outs_pool,
    ):
        identity = consts.tile([P, P], f32)
        make_identity(nc, identity[:])
        magic_ap = consts.tile([P, 1], f32, tag="magic", name="magic")
        nc.vector.memset(magic_ap[:], MAGIC)

        # PSUM: chunks 0-3 in psA banks 0-3, chunks 4-6 in psB banks 0-2;
        # psB bank 3 (the 8th PSUM bank) doubles as transpose/warmup scratch
        # (only touched before the first conv matmul).
        psA = psum_pool.tile([P, 4 * BANK], f32, tag="psA")
        psB = psum_pool.tile([P, 4 * BANK], f32, tag="psB")
        tp = psB[:, 3 * BANK:4 * BANK]

        # ---------------- DMA-in: x first (longest critical path) ----------
        xts = {}

        def issue_x(b):
            if b < bpc and b not in xts:
                xts[b] = acts.tile([P, NPIX], f32, tag="xt", name=f"xt{b}")
                nc.sync.dma_start(out=xts[b][:], in_=x_d[b])

        issue_x(0)
        wnat_h = wtmp.tile([P, P * NTAPS], f32, tag="wnat_h")
        wnat_l = wtmp.tile([P, P * NTAPS], f32, tag="wnat_l")
        nc.sync.dma_start(out=wnat_h[:], in_=wh_d)
        nc.sync.dma_start(out=wnat_l[:], in_=wl_d)

        # ---------------- per-image prep: mask -> bounds -> quantize --------
        qs = {}

        def prep(b):
            issue_x(b + 1)
            xt = xts[b]
            # w-blocksum written transposed so the h-blocksum reduce is a
            # single contiguous-group reduce
            r1 = acts.tile([P, H * NCHUNK], f32, tag="r1")
            nc.vector.reduce_sum(
                r1[:].rearrange("p (w h) -> p h w", w=NCHUNK),
                xt[:].rearrange("p (r c) -> p r c", c=POOL_K), axis=X)
            r2 = acts.tile([P, NCHUNK * NCHUNK], f32, tag="r2")
            nc.vector.reduce_sum(
                r2[:], r1[:].rearrange("p (g c) -> p g c", c=POOL_K), axis=X)
            mt = acts.tile([P, NCHUNK * NCHUNK], f32, tag="mt")
            nc.vector.tensor_scalar(mt[:], r2[:], sum_thresh, None, op0=op.is_ge)
            m = acts.tile([P, NCHUNK * NCHUNK], f32, tag="m")
            nc.vector.tensor_copy(
                out=m[:], in_=mt[:].rearrange("p (w h) -> p h w", w=NCHUNK))
            # clamp bounds in the MAGIC domain, at block resolution:
            # bound_h = MAGIC + 255*m ; bound_l = MAGIC + 15*(1-m)
            bh_blk = acts.tile([P, NCHUNK * NCHUNK], f32, tag="bh_blk")
            nc.vector.tensor_scalar(
                bh_blk[:], m[:], 255.0, MAGIC, op0=op.mult, op1=op.add)
            bl_blk = acts.tile([P, NCHUNK * NCHUNK], f32, tag="bl_blk")
            nc.vector.tensor_scalar(
                bl_blk[:], m[:], -15.0, MAGIC + 15.0, op0=op.mult, op1=op.add)
            bh = acts.tile([P, NCHUNK * W], f32, tag="bh")
            nc.vector.tensor_copy(
                out=bh[:].rearrange("p (g c) -> p g c", c=POOL_K),
                in_=bh_blk[:].unsqueeze(2).broadcast_to(
                    (P, NCHUNK * NCHUNK, POOL_K)))
            bl = acts.tile([P, NCHUNK * W], f32, tag="bl")
            nc.vector.tensor_copy(
                out=bl[:].rearrange("p (g c) -> p g c", c=POOL_K),
                in_=bl_blk[:].unsqueeze(2).broadcast_to(
                    (P, NCHUNK * NCHUNK, POOL_K)))

            def quant(conv, inv_s, bnd, qdt):
                # v = x/s + MAGIC (ACT); u = min(v, bound) (DVE, applies both
                # the qmax clip and the region mask); q = max(u,MAGIC)-MAGIC
                # cast to bf16/fp8 (DVE tensor_scalar, 2x mode)
                v = acts.tile([P, NPIX], f32, tag=f"v{conv}", name=f"v{conv}")
                nc.scalar.activation(
                    v[:], xt[:], IDENT, bias=magic_ap[:, 0:1], scale=inv_s)
                v4 = v[:].rearrange("p (hb r c) -> p hb r c", hb=NCHUNK, r=ROWS)
                bnd4 = (bnd[:].rearrange("p (hb c) -> p hb c", hb=NCHUNK)
                        .unsqueeze(2).broadcast_to((P, NCHUNK, ROWS, W)))
                nc.vector.tensor_tensor(v4, v4, bnd4, op=op.min)
                q = qtiles.tile([P, NPAD], qdt, tag=f"q{conv}", name=f"q{conv}")
                q3 = q[:].rearrange("p (r c) -> p r c", r=HP)
                nc.gpsimd.memset(q3[:, 0:HP:HP - 1, :], 0.0)
                nc.gpsimd.memset(q3[:, 1:HP - 1, 0:WP:WP - 1], 0.0)
                nc.vector.tensor_scalar(
                    q3[:, 1:H + 1, 1:W + 1],
                    v[:].rearrange("p (r c) -> p r c", r=H),
                    MAGIC, MAGIC, op0=op.max, op1=op.subtract)
                return q

            qs[b] = (quant("h", inv_sh, bh, bf16),
                     quant("l", inv_sl, bl, fp8))

        # ---------------- weight prep ----------------
        def pp(tag):
            return consts.tile([P, 1], f32, tag=tag, name=tag)

        am_h, am_l = pp("am_h"), pp("am_l")
        nc.vector.tensor_reduce(
            am_h[:], wnat_h[:], axis=X, op=op.max, apply_absolute_value=True)
        nc.vector.tensor_reduce(
            am_l[:], wnat_l[:], axis=X, op=op.max, apply_absolute_value=True)
        sv_l, sv_h = pp("sv_l"), pp("sv_h")
        nc.vector.tensor_scalar_mul(sv_l[:], am_l[:], c_svl)
        nc.vector.tensor_scalar_mul(sv_h[:], am_h[:], c_svh)
        rcp_svl, ratio = pp("rcp_svl"), pp("ratio")
        nc.vector.reciprocal(rcp_svl[:], sv_l[:])
        nc.vector.tensor_tensor(ratio[:], sv_h[:], rcp_svl[:], op=op.mult)
        rcp_h, rs_h = pp("rcp_h"), pp("rs_h")
        nc.vector.reciprocal(rcp_h[:], am_h[:])
        nc.vector.tensor_scalar_mul(rs_h[:], rcp_h[:], 127.0)
        rcp_l, rs_l = pp("rcp_l"), pp("rs_l")
        nc.vector.reciprocal(rcp_l[:], am_l[:])
        nc.vector.tensor_scalar_mul(rs_l[:], rcp_l[:], 7.0)

        # integer-quantize in natural [oc, ic*9] layout, in place
        nc.vector.tensor_scalar(
            wnat_h[:], wnat_h[:], rs_h[:, 0:1], MAGIC, op0=op.mult, op1=op.add)
        nc.vector.tensor_scalar(
            wnat_h[:], wnat_h[:], MAGIC, 127.0, op0=op.subtract, op1=op.min)
        # fold sv_h/sv_l into the high weights so both convs share one
        # output scale (sv_l); low weights stay exact fp8 integers
        nc.vector.tensor_scalar(
            wnat_h[:], wnat_h[:], -127.0, ratio[:, 0:1], op0=op.max, op1=op.mult)
        nc.vector.tensor_scalar(
            wnat_l[:], wnat_l[:], rs_l[:, 0:1], MAGIC, op0=op.mult, op1=op.add)
        nc.vector.tensor_scalar(
            wnat_l[:], wnat_l[:], MAGIC, 7.0, op0=op.subtract, op1=op.min)
        nc.vector.tensor_scalar_max(wnat_l[:], wnat_l[:], -7.0)

        # transpose each tap [oc, ic] -> [ic, oc]; cast to bf16 / fp8e4
        qwt_h = consts.tile([P, NTAPS * P], bf16, tag="qwt_h")
        qwt_l = consts.tile([P, NTAPS * P], fp8, tag="qwt_l")
        for wq, qwt in ((wnat_h, qwt_h), (wnat_l, qwt_l)):
            wv = wq[:].rearrange("p (i t) -> p t i", t=NTAPS)
            for base in range(0, NTAPS, 4):
                n = min(4, NTAPS - base)
                for j in range(n):
                    nc.tensor.transpose(
                        tp[:, j * P:(j + 1) * P], wv[:, base + j, :], identity[:])
                nc.vector.tensor_copy(
                    out=qwt[:, base * P:(base + n) * P], in_=tp[:, :n * P])

        # PE warm-up: HAM un-throttles after ~3.4us of sustained activity
        for i in range(28):
            nc.tensor.matmul(
                tp[:, 0:P], identity[:], identity[:],
                start=(i == 0), stop=(i == 27))

        prep(0)

        # ---------------- convs ----------------
        def seg_of(c):
            return (psA[:, c * BANK:c * BANK + NFREE] if c < 4
                    else psB[:, (c - 4) * BANK:(c - 4) * BANK + NFREE])

        def conv(b):
            # high phase first (needs only qh), then the fp8 low phase --
            # gives prep(b) maximal slack to finish ql while PE runs high.
            qh, ql = qs.pop(b)
            acc = outs_pool.tile([P, NPIX], f32, tag="acc")
            qh3 = qh[:].rearrange("p (r c) -> p r c", r=HP)
            ql3 = ql[:].rearrange("p (r c) -> p r c", r=HP)
            qlf = ql[:]
            for c in range(NCHUNK):
                r0 = c * ROWS
                seg = seg_of(c)
                for tap in range(NTAPS):
                    kh, kw = divmod(tap, 3)
                    nc.tensor.matmul(
                        seg, qwt_h[:, tap * P:(tap + 1) * P],
                        qh3[:, r0 + kh:r0 + kh + ROWS, kw:kw + W],
                        start=(tap == 0), stop=False)
            for c in range(NCHUNK):
                r0 = c * ROWS
                seg = seg_of(c)
                for pr in range(4):
                    t0 = 2 * pr
                    kh0, kw0 = divmod(t0, 3)
                    kh1, kw1 = divmod(t0 + 1, 3)
                    o0 = (r0 + kh0) * HP + kw0
                    ds = (r0 + kh1) * HP + kw1 - o0
                    # overlapping strided view [P, 2(pair), 8(rows), 56(cols)]
                    rv = (qlf[:, 0:2 * ROWS * W]
                          .rearrange("p (a b c) -> p a b c", a=2, b=ROWS)
                          .copy())
                    rv.ap = _br.VecI64Pair(
                        [[NPAD, P], [ds, 2], [HP, ROWS], [1, W]])
                    rv.offset = qlf.offset + o0
                    lhsT = (qwt_l[:, t0 * P:(t0 + 2) * P]
                            .rearrange("p (two m) -> p two m", two=2))
                    nc.tensor.matmul(
                        seg, lhsT, rv, start=False, stop=False, perf_mode=DR)
                nc.tensor.matmul(
                    seg, qwt_l[:, 8 * P:9 * P],
                    ql3[:, r0 + 2:r0 + 2 + ROWS, 2:2 + W],
                    start=False, stop=True)
                if c == 3:
                    nc.scalar.mul(
                        acc[:, 0:4 * NFREE].rearrange("p (b k) -> p b k", b=4),
                        psA[:].rearrange("p (b k) -> p b k", b=4)[:, :, 0:NFREE],
                        sv_l[:, 0:1])
                if c == 6:
                    nc.scalar.mul(
                        acc[:, 4 * NFREE:7 * NFREE]
                        .rearrange("p (b k) -> p b k", b=3),
                        psB[:, 0:3 * BANK]
                        .rearrange("p (b k) -> p b k", b=3)[:, :, 0:NFREE],
                        sv_l[:, 0:1])
            nc.sync.dma_start(out=y_d[b], in_=acc[:])

        for b in range(bpc):
            if b + 1 < bpc:
                prep(b + 1)
            conv(b)


def make_bass(inv_sh, inv_sl, c_svh, c_svl, bpc=BPC):
    import concourse.bacc as bacc
    import concourse.mybir as mybir
    from concourse.tile import TileContext

    f32 = mybir.dt.float32
    nc = bacc.Bacc("TRN2", debug=False)
    x = nc.dram_tensor("x", [bpc, P, NPIX], f32, kind="ExternalInput")
    wh = nc.dram_tensor("w_high", [P, P * NTAPS], f32, kind="ExternalInput")
    wl = nc.dram_tensor("w_low", [P, P * NTAPS], f32, kind="ExternalInput")
    y = nc.dram_tensor("y", [bpc, P, NPIX], f32, kind="ExternalOutput")
    aps = {"x": x.ap(), "w_high": wh.ap(), "w_low": wl.ap(), "y": y.ap()}
    with TileContext(nc) as tc:
        build_program(nc, tc, aps, inv_sh, inv_sl, c_svh, c_svl, bpc=bpc)
    nc.compile()
    return nc


def _scale_consts(act_scale_high, act_scale_low):
    sh = float(np.float32(act_scale_high))
    sl = float(np.float32(act_scale_low))
    inv_sh = float(np.float32(1.0 / np.float64(sh)))
    inv_sl = float(np.float32(1.0 / np.float64(sl)))
    c_svh = float(np.float32(np.float64(sh) / 127.0))
    c_svl = float(np.float32(np.float64(sl) / 7.0))
    return inv_sh, inv_sl, c_svh, c_svl


def _run(x, w_high, w_low, act_scale_high, act_scale_low, trace=False, **kw):
    from concourse import bass_utils

    x = np.ascontiguousarray(np.asarray(x, dtype=np.float32))
    w_high = np.ascontiguousarray(np.asarray(w_high, dtype=np.float32))
    w_low = np.ascontiguousarray(np.asarray(w_low, dtype=np.float32))

    inv_sh, inv_sl, c_svh, c_svl = _scale_consts(act_scale_high, act_scale_low)
    nc = make_bass(inv_sh, inv_sl, c_svh, c_svl)

    wh_flat = w_high.reshape(P, P * NTAPS)
    wl_flat = w_low.reshape(P, P * NTAPS)
    in_maps = []
    for core in range(N_CORES):
        xs = x[core * BPC:(core + 1) * BPC].reshape(BPC, P, NPIX)
        in_maps.append(
            {
                "x": np.ascontiguousarray(xs),
                "w_high": wh_flat,
                "w_low": wl_flat,
            }
        )
    res = bass_utils.run_bass_kernel_spmd(
        nc, in_maps, core_ids=list(range(N_CORES)), trace=trace, **kw
    )
    y = np.concatenate([r["y"].reshape(BPC, P, H, W) for r in res.results], axis=0)
    return y, res


def kernel(x, w_high, w_low, act_scale_high, act_scale_low):
    y, _ = _run(x, w_high, w_low, act_scale_high, act_scale_low)
    return y


# revision 15
# speedup vs baseline: 1.0516x; 1.0516x over previous
"""DRQConv2d (dual-region quantized conv) Trainium2 kernel — v2.

Reference semantics:
  mask  = upsample8(avgpool8(x) >= 0.05)             per (b, c)
  xh    = where(mask, x, 1e-5);  xl = where(mask, 1e-5, x)
  qh    = clip(round(xh/sh), 0, 255) * sh            (uint8 fake-quant)
  ql    = clip(round(xl/sl), 0, 15) * sl             (uint4 fake-quant)
  qwh   = per-oc quant of w_high to +-127,  qwl = per-oc quant of w_low to +-7
  y     = conv3x3(qh, qwh) + conv3x3(ql, qwl)        (pad 1)

Key ideas vs the v1 baseline (151us):
  * Low conv runs in fp8 e4m3 with MatmulPerfMode.DoubleRow: quantized low
    activations (ints 0..15) and weights (ints +-7) are exactly representable
    in e4m3, and DoubleRow packs TWO 3x3 taps (2x128 contraction rows) into
    one PE instruction -> 9 taps in 5 matmuls instead of 9.  The per-oc scale
    ratio sv_h/sv_l is folded into the HIGH (bf16) weights instead of the low
    ones so the low weights stay exact integers; both convs share PSUM banks
    and one final evacuation scale by sv_l (adds ~0.1% error, gate is 2%).
  * The region mask is applied as a CLAMP BOUND instead of a multiply:
    v = x/s + MAGIC (ACT, scale+round fused); u = min(v, MAGIC + qmax*mask)
    (GpSimd, mask expanded only to [P, 7*56] block-row resolution and
    broadcast-viewed); q = max(u, MAGIC) - MAGIC (DVE tensor_scalar, 2x mode).
    Masked-out pixels clamp to exactly MAGIC -> quantize to 0, which matches
    the reference (1e-5 rounds to 0).  This removes the full-res mask
    expansion and the 1x-mode scalar_tensor_tensor of v1.
  * PSUM is laid out as two bank-aligned supertiles (chunks at 512-elem
    stride) so each image needs only 2 strided ACT evacuations + 1 DMA out.

Sharding: data-parallel over batch. 32 images -> 4 per core on 8 cores,
weights replicated; outputs concatenated on host. No collectives.
"""

import numpy as np

P = 128            # channels (both in and out) == partitions
B_TOTAL = 32
N_CORES = 8
BPC = B_TOTAL // N_CORES   # images per core
H = W = 56
HP = WP = H + 2    # zero-padded layout
NPIX = H * W       # 3136
NPAD = HP * WP     # 3364
NTAPS = 9
ROWS = 8
NCHUNK = H // ROWS                    # 7
NFREE = ROWS * W                      # 448 columns per matmul
BANK = 512                            # PSUM bank stride in f32 elements
MAGIC = float(np.float32(1.5 * 2 ** 23))   # fp32 round-to-nearest magic
POOL_K = 8
THRESH = 0.05


def build_program(nc, tc, aps, inv_sh, inv_sl, c_svh, c_svl, bpc=BPC):
    import bass_rust as _br
    import concourse.mybir as mybir
    from concourse.alu_op_type import AluOpType as op
    from concourse.masks import make_identity

    f32 = mybir.dt.float32
    bf16 = mybir.dt.bfloat16
    fp8 = mybir.dt.float8e4
    X = mybir.AxisListType.X
    DR = mybir.MatmulPerfMode.DoubleRow
    IDENT = mybir.ActivationFunctionType.Identity

    x_d, wh_d, wl_d, y_d = aps["x"], aps["w_high"], aps["w_low"], aps["y"]
    sum_thresh = float(np.float32(THRESH) * POOL_K * POOL_K)  # exact pow2 scale

    with (
        tc.tile_pool(name="consts", bufs=1) as consts,
        tc.tile_pool(name="wtmp", bufs=1) as wtmp,
        tc.tile_pool(name="psum", bufs=1, space="PSUM") as psum_pool,
        tc.tile_pool(name="acts", bufs=2) as acts,
        tc.tile_pool(name="qtiles", bufs=3) as qtiles,
        tc.tile_pool(name="outs", bufs=2) as outs_pool,
    ):
        identity = consts.tile([P, P], f32)
        make_identity(nc, identity[:])
        magic_ap = consts.tile([P, 1], f32, tag="magic", name="magic")
        nc.vector.memset(magic_ap[:], MAGIC)
        nmagic_ap = consts.tile([P, 1], f32, tag="nmagic", name="nmagic")
        nc.vector.memset(nmagic_ap[:], -MAGIC)
        # touch the ACT table immediately so ACT_TABLE_LOAD is off the
        # critical path of the first quant pass
        warm_act = consts.tile([P, 1], f32, tag="warm_act", name="warm_act")
        nc.scalar.activation(warm_act[:], magic_ap[:], IDENT)

        # PSUM: chunks 0-3 in psA banks 0-3, chunks 4-6 in psB banks 0-2;
        # psB bank 3 (the 8th PSUM bank) doubles as transpose/warmup scratch
        # (only touched before the first conv matmul).
        psA = psum_pool.tile([P, 4 * BANK], f32, tag="psA")
        psB = psum_pool.tile([P, 4 * BANK], f32, tag="psB")
        tp = psB[:, 3 * BANK:4 * BANK]

        # ---------------- DMA-in: x first (longest critical path) ----------
        xts = {}

        def issue_x(b):
            if b < bpc and b not in xts:
                xts[b] = acts.tile([P, NPIX], f32, tag="xt", name=f"xt{b}")
                nc.sync.dma_start(out=xts[b][:], in_=x_d[b])

        wnat_h = wtmp.tile([P, P * NTAPS], f32, tag="wnat_h")
        wnat_l = wtmp.tile([P, P * NTAPS], f32, tag="wnat_l")
        nc.sync.dma_start(out=wnat_h[:], in_=wh_d)
        nc.sync.dma_start(out=wnat_l[:], in_=wl_d)
        issue_x(0)

        # ---------------- per-image prep: mask -> bounds -> quantize --------
        qs = {}

        def prep(b):
            issue_x(b + 1)
            xt = xts[b]
            # w-blocksum written transposed so the h-blocksum reduce is a
            # single contiguous-group reduce
            r1 = acts.tile([P, H * NCHUNK], f32, tag="r1")
            nc.vector.reduce_sum(
                r1[:].rearrange("p (w h) -> p h w", w=NCHUNK),
                xt[:].rearrange("p (r c) -> p r c", c=POOL_K), axis=X)
            r2 = acts.tile([P, NCHUNK * NCHUNK], f32, tag="r2")
            nc.vector.reduce_sum(
                r2[:], r1[:].rearrange("p (g c) -> p g c", c=POOL_K), axis=X)
            mt = acts.tile([P, NCHUNK * NCHUNK], f32, tag="mt")
            nc.vector.tensor_scalar(mt[:], r2[:], sum_thresh, None, op0=op.is_ge)
            m = acts.tile([P, NCHUNK * NCHUNK], f32, tag="m")
            nc.vector.tensor_copy(
                out=m[:], in_=mt[:].rearrange("p (w h) -> p h w", w=NCHUNK))
            # clamp bounds in the MAGIC domain, at block resolution:
            # bound_h = MAGIC + 255*m ; bound_l = MAGIC + 15*(1-m)
            bh_blk = acts.tile([P, NCHUNK * NCHUNK], f32, tag="bh_blk")
            nc.vector.tensor_scalar(
                bh_blk[:], m[:], 255.0, MAGIC, op0=op.mult, op1=op.add)
            bl_blk = acts.tile([P, NCHUNK * NCHUNK], f32, tag="bl_blk")
            nc.vector.tensor_scalar(
                bl_blk[:], m[:], -15.0, MAGIC + 15.0, op0=op.mult, op1=op.add)
            bh = acts.tile([P, NCHUNK * W], f32, tag="bh")
            nc.vector.tensor_copy(
                out=bh[:].rearrange("p (g c) -> p g c", c=POOL_K),
                in_=bh_blk[:].unsqueeze(2).broadcast_to(
                    (P, NCHUNK * NCHUNK, POOL_K)))
            bl = acts.tile([P, NCHUNK * W], f32, tag="bl")
            nc.vector.tensor_copy(
                out=bl[:].rearrange("p (g c) -> p g c", c=POOL_K),
                in_=bl_blk[:].unsqueeze(2).broadcast_to(
                    (P, NCHUNK * NCHUNK, POOL_K)))

            def quant(conv, inv_s, bnd, qdt):
                # v = x/s + MAGIC (ACT); u = min(v, bound) (DVE, applies both
                # the qmax clip and the region mask); q = max(u,MAGIC)-MAGIC
                # cast to bf16/fp8 (DVE tensor_scalar, 2x mode)
                v = acts.tile([P, NPIX], f32, tag=f"v{conv}", name=f"v{conv}")
                nc.scalar.activation(
                    v[:], xt[:], IDENT, bias=magic_ap[:, 0:1], scale=inv_s)
                v4 = v[:].rearrange("p (hb r c) -> p hb r c", hb=NCHUNK, r=ROWS)
                bnd4 = (bnd[:].rearrange("p (hb c) -> p hb c", hb=NCHUNK)
                        .unsqueeze(2).broadcast_to((P, NCHUNK, ROWS, W)))
                nc.vector.tensor_tensor(v4, v4, bnd4, op=op.min)
                q = qtiles.tile([P, NPAD], qdt, tag=f"q{conv}", name=f"q{conv}")
                q3 = q[:].rearrange("p (r c) -> p r c", r=HP)
                nc.gpsimd.memset(q3[:, 0:HP:HP - 1, :], 0.0)
                nc.gpsimd.memset(q3[:, 1:HP - 1, 0:WP:WP - 1], 0.0)
                nc.vector.tensor_scalar(
                    q3[:, 1:H + 1, 1:W + 1],
                    v[:].rearrange("p (r c) -> p r c", r=H),
                    MAGIC, MAGIC, op0=op.max, op1=op.subtract)
                return q

            qs[b] = (quant("h", inv_sh, bh, bf16),
                     quant("l", inv_sl, bl, fp8))

        # ---------------- weight prep ----------------
        def pp(tag):
            return consts.tile([P, 1], f32, tag=tag, name=tag)

        am_h, am_l = pp("am_h"), pp("am_l")
        nc.vector.tensor_reduce(
            am_h[:], wnat_h[:], axis=X, op=op.max, apply_absolute_value=True)
        nc.vector.tensor_reduce(
            am_l[:], wnat_l[:], axis=X, op=op.max, apply_absolute_value=True)
        sv_l, sv_h = pp("sv_l"), pp("sv_h")
        nc.vector.tensor_scalar_mul(sv_l[:], am_l[:], c_svl)
        nc.vector.tensor_scalar_mul(sv_h[:], am_h[:], c_svh)
        rcp_svl, ratio = pp("rcp_svl"), pp("ratio")
        nc.vector.reciprocal(rcp_svl[:], sv_l[:])
        nc.vector.tensor_tensor(ratio[:], sv_h[:], rcp_svl[:], op=op.mult)
        rcp_h, rs_h = pp("rcp_h"), pp("rs_h")
        nc.vector.reciprocal(rcp_h[:], am_h[:])
        nc.vector.tensor_scalar_mul(rs_h[:], rcp_h[:], 127.0)
        rcp_l, rs_l = pp("rcp_l"), pp("rs_l")
        nc.vector.reciprocal(rcp_l[:], am_l[:])
        nc.vector.tensor_scalar_mul(rs_l[:], rcp_l[:], 7.0)

        # integer-quantize in natural [oc, ic*9] layout: the magic round
        # runs on the (otherwise idle) ACT engine -- a1 = w*rs + MAGIC,
        # a2 = a1 - MAGIC (Sterbenz-exact) -- and DVE only clamps/scales.
        nc.scalar.activation(
            wnat_h[:], wnat_h[:], IDENT, bias=magic_ap[:, 0:1],
            scale=rs_h[:, 0:1])
        nc.scalar.activation(
            wnat_h[:], wnat_h[:], IDENT, bias=nmagic_ap[:, 0:1], scale=1.0)
        # fold sv_h/sv_l into the high weights so both convs share one
        # output scale (sv_l); low weights stay exact fp8 integers
        nc.vector.tensor_scalar(
            wnat_h[:], wnat_h[:], 127.0, -127.0, op0=op.min, op1=op.max)
        nc.vector.tensor_scalar_mul(wnat_h[:], wnat_h[:], ratio[:, 0:1])
        nc.scalar.activation(
            wnat_l[:], wnat_l[:], IDENT, bias=magic_ap[:, 0:1],
            scale=rs_l[:, 0:1])
        nc.scalar.activation(
            wnat_l[:], wnat_l[:], IDENT, bias=nmagic_ap[:, 0:1], scale=1.0)
        nc.vector.tensor_scalar(
            wnat_l[:], wnat_l[:], 7.0, -7.0, op0=op.min, op1=op.max)

        # transpose each tap [oc, ic] -> [ic, oc]; cast to bf16 / fp8e4
        qwt_h = consts.tile([P, NTAPS * P], bf16, tag="qwt_h")
        qwt_l = consts.tile([P, NTAPS * P], fp8, tag="qwt_l")
        for wq, qwt in ((wnat_h, qwt_h), (wnat_l, qwt_l)):
            wv = wq[:].rearrange("p (i t) -> p t i", t=NTAPS)
            for base in range(0, NTAPS, 4):
                n = min(4, NTAPS - base)
                for j in range(n):
                    nc.tensor.transpose(
                        tp[:, j * P:(j + 1) * P], wv[:, base + j, :], identity[:])
                nc.vector.tensor_copy(
                    out=qwt[:, base * P:(base + n) * P], in_=tp[:, :n * P])

        # PE warm-up: HAM un-throttles after ~3.4us of sustained activity
        for i in range(28):
            nc.tensor.matmul(
                tp[:, 0:P], identity[:], identity[:],
                start=(i == 0), stop=(i == 27))

        prep(0)

        # ---------------- convs ----------------
        def seg_of(c):
            return (psA[:, c * BANK:c * BANK + NFREE] if c < 4
                    else psB[:, (c - 4) * BANK:(c - 4) * BANK + NFREE])

        def conv(b):
            # high phase first (needs only qh), then the fp8 low phase --
            # gives prep(b) maximal slack to finish ql while PE runs high.
            qh, ql = qs.pop(b)
            acc = outs_pool.tile([P, NPIX], f32, tag="acc")
            qh3 = qh[:].rearrange("p (r c) -> p r c", r=HP)
            ql3 = ql[:].rearrange("p (r c) -> p r c", r=HP)
            qlf = ql[:]
            for c in range(NCHUNK):
                r0 = c * ROWS
                seg = seg_of(c)
                for tap in range(NTAPS):
                    kh, kw = divmod(tap, 3)
                    nc.tensor.matmul(
                        seg, qwt_h[:, tap * P:(tap + 1) * P],
                        qh3[:, r0 + kh:r0 + kh + ROWS, kw:kw + W],
                        start=(tap == 0), stop=False)
            for c in range(NCHUNK):
                r0 = c * ROWS
                seg = seg_of(c)
                for pr in range(4):
                    t0 = 2 * pr
                    kh0, kw0 = divmod(t0, 3)
                    kh1, kw1 = divmod(t0 + 1, 3)
                    o0 = (r0 + kh0) * HP + kw0
                    ds = (r0 + kh1) * HP + kw1 - o0
                    # overlapping strided view [P, 2(pair), 8(rows), 56(cols)]
                    rv = (qlf[:, 0:2 * ROWS * W]
                          .rearrange("p (a b c) -> p a b c", a=2, b=ROWS)
                          .copy())
                    rv.ap = _br.VecI64Pair(
                        [[NPAD, P], [ds, 2], [HP, ROWS], [1, W]])
                    rv.offset = qlf.offset + o0
                    lhsT = (qwt_l[:, t0 * P:(t0 + 2) * P]
                            .rearrange("p (two m) -> p two m", two=2))
                    nc.tensor.matmul(
                        seg, lhsT, rv, start=False, stop=False, perf_mode=DR)
                nc.tensor.matmul(
                    seg, qwt_l[:, 8 * P:9 * P],
                    ql3[:, r0 + 2:r0 + 2 + ROWS, 2:2 + W],
                    start=False, stop=True)
                if c == 3:
                    nc.scalar.mul(
                        acc[:, 0:4 * NFREE].rearrange("p (b k) -> p b k", b=4),
                        psA[:].rearrange("p (b k) -> p b k", b=4)[:, :, 0:NFREE],
                        sv_l[:, 0:1])
                    nc.sync.dma_start(
                        out=y_d[b][:, 0:4 * NFREE], in_=acc[:, 0:4 * NFREE])
                if c == 6:
                    nc.scalar.mul(
                        acc[:, 4 * NFREE:7 * NFREE]
                        .rearrange("p (b k) -> p b k", b=3),
                        psB[:, 0:3 * BANK]
                        .rearrange("p (b k) -> p b k", b=3)[:, :, 0:NFREE],
                        sv_l[:, 0:1])
                    nc.sync.dma_start(
                        out=y_d[b][:, 4 * NFREE:7 * NFREE],
                        in_=acc[:, 4 * NFREE:7 * NFREE])

        for b in range(bpc):
            if b + 1 < bpc:
                prep(b + 1)
            conv(b)


def make_bass(inv_sh, inv_sl, c_svh, c_svl, bpc=BPC):
    import concourse.bacc as bacc
    import concourse.mybir as mybir
    from concourse.tile import TileContext

    f32 = mybir.dt.float32
    nc = bacc.Bacc("TRN2", debug=False)
    x = nc.dram_tensor("x", [bpc, P, NPIX], f32, kind="ExternalInput")
    wh = nc.dram_tensor("w_high", [P, P * NTAPS], f32, kind="ExternalInput")
    wl = nc.dram_tensor("w_low", [P, P * NTAPS], f32, kind="ExternalInput")
    y = nc.dram_tensor("y", [bpc, P, NPIX], f32, kind="ExternalOutput")
    aps = {"x": x.ap(), "w_high": wh.ap(), "w_low": wl.ap(), "y": y.ap()}
    with TileContext(nc) as tc:
        build_program(nc, tc, aps, inv_sh, inv_sl, c_svh, c_svl, bpc=bpc)
    nc.compile()
    return nc


def _scale_consts(act_scale_high, act_scale_low):
    sh = float(np.float32(act_scale_high))
    sl = float(np.float32(act_scale_low))
    inv_sh = float(np.float32(1.0 / np.float64(sh)))
    inv_sl = float(np.float32(1.0 / np.float64(sl)))
    c_svh = float(np.float32(np.float64(sh) / 127.0))
    c_svl = float(np.float32(np.float64(sl) / 7.0))
    return inv_sh, inv_sl, c_svh, c_svl


def _run(x, w_high, w_low, act_scale_high, act_scale_low, trace=False, **kw):
    from concourse import bass_utils

    x = np.ascontiguousarray(np.asarray(x, dtype=np.float32))
    w_high = np.ascontiguousarray(np.asarray(w_high, dtype=np.float32))
    w_low = np.ascontiguousarray(np.asarray(w_low, dtype=np.float32))

    inv_sh, inv_sl, c_svh, c_svl = _scale_consts(act_scale_high, act_scale_low)
    nc = make_bass(inv_sh, inv_sl, c_svh, c_svl)

    wh_flat = w_high.reshape(P, P * NTAPS)
    wl_flat = w_low.reshape(P, P * NTAPS)
    in_maps = []
    for core in range(N_CORES):
        xs = x[core * BPC:(core + 1) * BPC].reshape(BPC, P, NPIX)
        in_maps.append(
            {
                "x": np.ascontiguousarray(xs),
                "w_high": wh_flat,
                "w_low": wl_flat,
            }
        )
    res = bass_utils.run_bass_kernel_spmd(
        nc, in_maps, core_ids=list(range(N_CORES)), trace=trace, **kw
    )
    y = np.concatenate([r["y"].reshape(BPC, P, H, W) for r in res.results], axis=0)
    return y, res


def kernel(x, w_high, w_low, act_scale_high, act_scale_low):
    y, _ = _run(x, w_high, w_low, act_scale_high, act_scale_low)
    return y


# revision 16
# speedup vs baseline: 1.0690x; 1.0165x over previous
"""DRQConv2d (dual-region quantized conv) Trainium2 kernel — v2.

Reference semantics:
  mask  = upsample8(avgpool8(x) >= 0.05)             per (b, c)
  xh    = where(mask, x, 1e-5);  xl = where(mask, 1e-5, x)
  qh    = clip(round(xh/sh), 0, 255) * sh            (uint8 fake-quant)
  ql    = clip(round(xl/sl), 0, 15) * sl             (uint4 fake-quant)
  qwh   = per-oc quant of w_high to +-127,  qwl = per-oc quant of w_low to +-7
  y     = conv3x3(qh, qwh) + conv3x3(ql, qwl)        (pad 1)

Key ideas vs the v1 baseline (151us):
  * Low conv runs in fp8 e4m3 with MatmulPerfMode.DoubleRow: quantized low
    activations (ints 0..15) and weights (ints +-7) are exactly representable
    in e4m3, and DoubleRow packs TWO 3x3 taps (2x128 contraction rows) into
    one PE instruction -> 9 taps in 5 matmuls instead of 9.  The per-oc scale
    ratio sv_h/sv_l is folded into the HIGH (bf16) weights instead of the low
    ones so the low weights stay exact integers; both convs share PSUM banks
    and one final evacuation scale by sv_l (adds ~0.1% error, gate is 2%).
  * The region mask is applied as a CLAMP BOUND instead of a multiply:
    v = x/s + MAGIC (ACT, scale+round fused); u = min(v, MAGIC + qmax*mask)
    (GpSimd, mask expanded only to [P, 7*56] block-row resolution and
    broadcast-viewed); q = max(u, MAGIC) - MAGIC (DVE tensor_scalar, 2x mode).
    Masked-out pixels clamp to exactly MAGIC -> quantize to 0, which matches
    the reference (1e-5 rounds to 0).  This removes the full-res mask
    expansion and the 1x-mode scalar_tensor_tensor of v1.
  * PSUM is laid out as two bank-aligned supertiles (chunks at 512-elem
    stride) so each image needs only 2 strided ACT evacuations + 1 DMA out.

Sharding: data-parallel over batch. 32 images -> 4 per core on 8 cores,
weights replicated; outputs concatenated on host. No collectives.
"""

import numpy as np

P = 128            # channels (both in and out) == partitions
B_TOTAL = 32
N_CORES = 8
BPC = B_TOTAL // N_CORES   # images per core
H = W = 56
HP = WP = H + 2    # zero-padded layout
NPIX = H * W       # 3136
NPAD = HP * WP     # 3364
NTAPS = 9
ROWS = 8
NCHUNK = H // ROWS                    # 7
NFREE = ROWS * W                      # 448 columns per matmul
BANK = 512                            # PSUM bank stride in f32 elements
MAGIC = float(np.float32(1.5 * 2 ** 23))   # fp32 round-to-nearest magic
POOL_K = 8
THRESH = 0.05


def build_program(nc, tc, aps, inv_sh, inv_sl, c_svh, c_svl, bpc=BPC):
    import bass_rust as _br
    import concourse.mybir as mybir
    from concourse.alu_op_type import AluOpType as op
    from concourse.masks import make_identity

    f32 = mybir.dt.float32
    bf16 = mybir.dt.bfloat16
    fp8 = mybir.dt.float8e4
    X = mybir.AxisListType.X
    DR = mybir.MatmulPerfMode.DoubleRow
    IDENT = mybir.ActivationFunctionType.Identity

    x_d, wh_d, wl_d, y_d = aps["x"], aps["w_high"], aps["w_low"], aps["y"]
    sum_thresh = float(np.float32(THRESH) * POOL_K * POOL_K)  # exact pow2 scale

    with (
        tc.tile_pool(name="consts", bufs=1) as consts,
        tc.tile_pool(name="wtmp", bufs=1) as wtmp,
        tc.tile_pool(name="psum", bufs=1, space="PSUM") as psum_pool,
        tc.tile_pool(name="acts", bufs=2) as acts,
        tc.tile_pool(name="qtiles", bufs=3) as qtiles,
        tc.tile_pool(name="outs", bufs=2) as outs_pool,
    ):
        identity = consts.tile([P, P], f32)
        make_identity(nc, identity[:])
        magic_ap = consts.tile([P, 1], f32, tag="magic", name="magic")
        nc.vector.memset(magic_ap[:], MAGIC)
        nmagic_ap = consts.tile([P, 1], f32, tag="nmagic", name="nmagic")
        nc.vector.memset(nmagic_ap[:], -MAGIC)
        # touch the ACT table immediately so ACT_TABLE_LOAD is off the
        # critical path of the first quant pass
        warm_act = consts.tile([P, 1], f32, tag="warm_act", name="warm_act")
        nc.scalar.activation(warm_act[:], magic_ap[:], IDENT)

        # PSUM: chunks 0-3 in psA banks 0-3, chunks 4-6 in psB banks 0-2;
        # psB bank 3 (the 8th PSUM bank) doubles as transpose/warmup scratch
        # (only touched before the first conv matmul).
        psA = psum_pool.tile([P, 4 * BANK], f32, tag="psA")
        psB = psum_pool.tile([P, 4 * BANK], f32, tag="psB")
        tp = psB[:, 3 * BANK:4 * BANK]

        # ---------------- DMA-in: x first (longest critical path) ----------
        xts = {}

        def issue_x(b):
            if b < bpc and b not in xts:
                xts[b] = acts.tile([P, NPIX], f32, tag="xt", name=f"xt{b}")
                nc.sync.dma_start(out=xts[b][:], in_=x_d[b])

        wnat_h = wtmp.tile([P, P * NTAPS], f32, tag="wnat_h")
        wnat_l = wtmp.tile([P, P * NTAPS], f32, tag="wnat_l")
        nc.sync.dma_start(out=wnat_h[:], in_=wh_d)
        nc.sync.dma_start(out=wnat_l[:], in_=wl_d)
        issue_x(0)

        # ---------------- per-image prep: mask -> bounds -> quantize --------
        qs = {}

        def prep(b):
            issue_x(b + 1)
            xt = xts[b]
            # w-blocksum written transposed so the h-blocksum reduce is a
            # single contiguous-group reduce
            r1 = acts.tile([P, H * NCHUNK], f32, tag="r1")
            nc.vector.reduce_sum(
                r1[:].rearrange("p (w h) -> p h w", w=NCHUNK),
                xt[:].rearrange("p (r c) -> p r c", c=POOL_K), axis=X)
            r2 = acts.tile([P, NCHUNK * NCHUNK], f32, tag="r2")
            nc.vector.reduce_sum(
                r2[:], r1[:].rearrange("p (g c) -> p g c", c=POOL_K), axis=X)
            mt = acts.tile([P, NCHUNK * NCHUNK], f32, tag="mt")
            nc.vector.tensor_scalar(mt[:], r2[:], sum_thresh, None, op0=op.is_ge)
            m = acts.tile([P, NCHUNK * NCHUNK], f32, tag="m")
            nc.vector.tensor_copy(
                out=m[:], in_=mt[:].rearrange("p (w h) -> p h w", w=NCHUNK))
            # clamp bounds in the MAGIC domain, at block resolution:
            # bound_h = MAGIC + 255*m ; bound_l = MAGIC + 15*(1-m)
            bh_blk = acts.tile([P, NCHUNK * NCHUNK], f32, tag="bh_blk")
            nc.vector.tensor_scalar(
                bh_blk[:], m[:], 255.0, MAGIC, op0=op.mult, op1=op.add)
            bl_blk = acts.tile([P, NCHUNK * NCHUNK], f32, tag="bl_blk")
            nc.vector.tensor_scalar(
                bl_blk[:], m[:], -15.0, MAGIC + 15.0, op0=op.mult, op1=op.add)
            bh = acts.tile([P, NCHUNK * W], f32, tag="bh")
            nc.vector.tensor_copy(
                out=bh[:].rearrange("p (g c) -> p g c", c=POOL_K),
                in_=bh_blk[:].unsqueeze(2).broadcast_to(
                    (P, NCHUNK * NCHUNK, POOL_K)))
            bl = acts.tile([P, NCHUNK * W], f32, tag="bl")
            nc.vector.tensor_copy(
                out=bl[:].rearrange("p (g c) -> p g c", c=POOL_K),
                in_=bl_blk[:].unsqueeze(2).broadcast_to(
                    (P, NCHUNK * NCHUNK, POOL_K)))

            def quant(conv, inv_s, bnd, qdt):
                # v = x/s + MAGIC (ACT); u = min(v, bound) (DVE, applies both
                # the qmax clip and the region mask); q = max(u,MAGIC)-MAGIC
                # cast to bf16/fp8 (DVE tensor_scalar, 2x mode)
                v = acts.tile([P, NPIX], f32, tag=f"v{conv}", name=f"v{conv}")
                nc.scalar.activation(
                    v[:], xt[:], IDENT, bias=magic_ap[:, 0:1], scale=inv_s)
                v4 = v[:].rearrange("p (hb r c) -> p hb r c", hb=NCHUNK, r=ROWS)
                bnd4 = (bnd[:].rearrange("p (hb c) -> p hb c", hb=NCHUNK)
                        .unsqueeze(2).broadcast_to((P, NCHUNK, ROWS, W)))
                nc.vector.tensor_tensor(v4, v4, bnd4, op=op.min)
                q = qtiles.tile([P, NPAD], qdt, tag=f"q{conv}", name=f"q{conv}")
                q3 = q[:].rearrange("p (r c) -> p r c", r=HP)
                nc.gpsimd.memset(q3[:, 0:HP:HP - 1, :], 0.0)
                nc.gpsimd.memset(q3[:, 1:HP - 1, 0:WP:WP - 1], 0.0)
                nc.vector.tensor_scalar(
                    q3[:, 1:H + 1, 1:W + 1],
                    v[:].rearrange("p (r c) -> p r c", r=H),
                    MAGIC, MAGIC, op0=op.max, op1=op.subtract)
                return q

            qs[b] = (quant("h", inv_sh, bh, bf16),
                     quant("l", inv_sl, bl, fp8))

        # ---------------- weight prep ----------------
        def pp(tag):
            return consts.tile([P, 1], f32, tag=tag, name=tag)

        am_h, am_l = pp("am_h"), pp("am_l")
        nc.vector.tensor_reduce(
            am_h[:], wnat_h[:], axis=X, op=op.max, apply_absolute_value=True)
        nc.vector.tensor_reduce(
            am_l[:], wnat_l[:], axis=X, op=op.max, apply_absolute_value=True)
        sv_l, sv_h = pp("sv_l"), pp("sv_h")
        nc.vector.tensor_scalar_mul(sv_l[:], am_l[:], c_svl)
        nc.vector.tensor_scalar_mul(sv_h[:], am_h[:], c_svh)
        rcp_svl, ratio = pp("rcp_svl"), pp("ratio")
        nc.vector.reciprocal(rcp_svl[:], sv_l[:])
        nc.vector.tensor_tensor(ratio[:], sv_h[:], rcp_svl[:], op=op.mult)
        rcp_h, rs_h = pp("rcp_h"), pp("rs_h")
        nc.vector.reciprocal(rcp_h[:], am_h[:])
        nc.vector.tensor_scalar_mul(rs_h[:], rcp_h[:], 127.0)
        rcp_l, rs_l = pp("rcp_l"), pp("rs_l")
        nc.vector.reciprocal(rcp_l[:], am_l[:])
        nc.vector.tensor_scalar_mul(rs_l[:], rcp_l[:], 7.0)

        # integer-quantize in natural [oc, ic*9] layout: the magic round
        # runs on the (otherwise idle) ACT engine -- a1 = w*rs + MAGIC,
        # a2 = a1 - MAGIC (Sterbenz-exact) -- and DVE only clamps/scales.
        nc.scalar.activation(
            wnat_h[:], wnat_h[:], IDENT, bias=magic_ap[:, 0:1],
            scale=rs_h[:, 0:1])
        nc.scalar.activation(
            wnat_h[:], wnat_h[:], IDENT, bias=nmagic_ap[:, 0:1], scale=1.0)
        # fold sv_h/sv_l into the high weights so both convs share one
        # output scale (sv_l); low weights stay exact fp8 integers
        nc.vector.tensor_scalar(
            wnat_h[:], wnat_h[:], 127.0, -127.0, op0=op.min, op1=op.max)
        nc.vector.tensor_scalar_mul(wnat_h[:], wnat_h[:], ratio[:, 0:1])
        nc.scalar.activation(
            wnat_l[:], wnat_l[:], IDENT, bias=magic_ap[:, 0:1],
            scale=rs_l[:, 0:1])
        nc.scalar.activation(
            wnat_l[:], wnat_l[:], IDENT, bias=nmagic_ap[:, 0:1], scale=1.0)
        nc.vector.tensor_scalar(
            wnat_l[:], wnat_l[:], 7.0, -7.0, op0=op.min, op1=op.max)

        # transpose each tap [oc, ic] -> [ic, oc]; cast to bf16 / fp8e4
        qwt_h = consts.tile([P, NTAPS * P], bf16, tag="qwt_h")
        qwt_l = consts.tile([P, NTAPS * P], fp8, tag="qwt_l")
        for wq, qwt in ((wnat_h, qwt_h), (wnat_l, qwt_l)):
            wv = wq[:].rearrange("p (i t) -> p t i", t=NTAPS)
            for base in range(0, NTAPS, 4):
                n = min(4, NTAPS - base)
                for j in range(n):
                    nc.tensor.transpose(
                        tp[:, j * P:(j + 1) * P], wv[:, base + j, :], identity[:])
                # PSUM->SBUF evacuation on ACT (idle at startup); keeping it
                # off the DVE queue unblocks the transpose->warmup->conv chain
                nc.scalar.copy(
                    out=qwt[:, base * P:(base + n) * P], in_=tp[:, :n * P])

        # PE warm-up: HAM un-throttles after ~3.4us of sustained activity
        for i in range(28):
            nc.tensor.matmul(
                tp[:, 0:P], identity[:], identity[:],
                start=(i == 0), stop=(i == 27))

        prep(0)

        # ---------------- convs ----------------
        def seg_of(c):
            return (psA[:, c * BANK:c * BANK + NFREE] if c < 4
                    else psB[:, (c - 4) * BANK:(c - 4) * BANK + NFREE])

        def conv(b):
            # high phase first (needs only qh), then the fp8 low phase --
            # gives prep(b) maximal slack to finish ql while PE runs high.
            qh, ql = qs.pop(b)
            acc = outs_pool.tile([P, NPIX], f32, tag="acc")
            qh3 = qh[:].rearrange("p (r c) -> p r c", r=HP)
            ql3 = ql[:].rearrange("p (r c) -> p r c", r=HP)
            qlf = ql[:]
            for c in range(NCHUNK):
                r0 = c * ROWS
                seg = seg_of(c)
                for tap in range(NTAPS):
                    kh, kw = divmod(tap, 3)
                    nc.tensor.matmul(
                        seg, qwt_h[:, tap * P:(tap + 1) * P],
                        qh3[:, r0 + kh:r0 + kh + ROWS, kw:kw + W],
                        start=(tap == 0), stop=False)
            for c in range(NCHUNK):
                r0 = c * ROWS
                seg = seg_of(c)
                for pr in range(4):
                    t0 = 2 * pr
                    kh0, kw0 = divmod(t0, 3)
                    kh1, kw1 = divmod(t0 + 1, 3)
                    o0 = (r0 + kh0) * HP + kw0
                    ds = (r0 + kh1) * HP + kw1 - o0
                    # overlapping strided view [P, 2(pair), 8(rows), 56(cols)]
                    rv = (qlf[:, 0:2 * ROWS * W]
                          .rearrange("p (a b c) -> p a b c", a=2, b=ROWS)
                          .copy())
                    rv.ap = _br.VecI64Pair(
                        [[NPAD, P], [ds, 2], [HP, ROWS], [1, W]])
                    rv.offset = qlf.offset + o0
                    lhsT = (qwt_l[:, t0 * P:(t0 + 2) * P]
                            .rearrange("p (two m) -> p two m", two=2))
                    nc.tensor.matmul(
                        seg, lhsT, rv, start=False, stop=False, perf_mode=DR)
                nc.tensor.matmul(
                    seg, qwt_l[:, 8 * P:9 * P],
                    ql3[:, r0 + 2:r0 + 2 + ROWS, 2:2 + W],
                    start=False, stop=True)
                if c == 3:
                    nc.scalar.mul(
                        acc[:, 0:4 * NFREE].rearrange("p (b k) -> p b k", b=4),
                        psA[:].rearrange("p (b k) -> p b k", b=4)[:, :, 0:NFREE],
                        sv_l[:, 0:1])
                    nc.sync.dma_start(
                        out=y_d[b][:, 0:4 * NFREE], in_=acc[:, 0:4 * NFREE])
                if c == 6:
                    nc.scalar.mul(
                        acc[:, 4 * NFREE:7 * NFREE]
                        .rearrange("p (b k) -> p b k", b=3),
                        psB[:, 0:3 * BANK]
                        .rearrange("p (b k) -> p b k", b=3)[:, :, 0:NFREE],
                        sv_l[:, 0:1])
                    nc.sync.dma_start(
                        out=y_d[b][:, 4 * NFREE:7 * NFREE],
                        in_=acc[:, 4 * NFREE:7 * NFREE])

        for b in range(bpc):
            if b + 1 < bpc:
                prep(b + 1)
            conv(b)


def make_bass(inv_sh, inv_sl, c_svh, c_svl, bpc=BPC):
    import concourse.bacc as bacc
    import concourse.mybir as mybir
    from concourse.tile import TileContext

    f32 = mybir.dt.float32
    nc = bacc.Bacc("TRN2", debug=False)
    x = nc.dram_tensor("x", [bpc, P, NPIX], f32, kind="ExternalInput")
    wh = nc.dram_tensor("w_high", [P, P * NTAPS], f32, kind="ExternalInput")
    wl = nc.dram_tensor("w_low", [P, P * NTAPS], f32, kind="ExternalInput")
    y = nc.dram_tensor("y", [bpc, P, NPIX], f32, kind="ExternalOutput")
    aps = {"x": x.ap(), "w_high": wh.ap(), "w_low": wl.ap(), "y": y.ap()}
    with TileContext(nc) as tc:
        build_program(nc, tc, aps, inv_sh, inv_sl, c_svh, c_svl, bpc=bpc)
    nc.compile()
    return nc


def _scale_consts(act_scale_high, act_scale_low):
    sh = float(np.float32(act_scale_high))
    sl = float(np.float32(act_scale_low))
    inv_sh = float(np.float32(1.0 / np.float64(sh)))
    inv_sl = float(np.float32(1.0 / np.float64(sl)))
    c_svh = float(np.float32(np.float64(sh) / 127.0))
    c_svl = float(np.float32(np.float64(sl) / 7.0))
    return inv_sh, inv_sl, c_svh, c_svl


def _run(x, w_high, w_low, act_scale_high, act_scale_low, trace=False, **kw):
    from concourse import bass_utils

    x = np.ascontiguousarray(np.asarray(x, dtype=np.float32))
    w_high = np.ascontiguousarray(np.asarray(w_high, dtype=np.float32))
    w_low = np.ascontiguousarray(np.asarray(w_low, dtype=np.float32))

    inv_sh, inv_sl, c_svh, c_svl = _scale_consts(act_scale_high, act_scale_low)
    nc = make_bass(inv_sh, inv_sl, c_svh, c_svl)

    wh_flat = w_high.reshape(P, P * NTAPS)
    wl_flat = w_low.reshape(P, P * NTAPS)
    in_maps = []
    for core in range(N_CORES):
        xs = x[core * BPC:(core + 1) * BPC].reshape(BPC, P, NPIX)
        in_maps.append(
            {
                "x": np.ascontiguousarray(xs),
                "w_high": wh_flat,
                "w_low": wl_flat,
            }
        )
    res = bass_utils.run_bass_kernel_spmd(
        nc, in_maps, core_ids=list(range(N_CORES)), trace=trace, **kw
    )
    y = np.concatenate([r["y"].reshape(BPC, P, H, W) for r in res.results], axis=0)
    return y, res


def kernel(x, w_high, w_low, act_scale_high, act_scale_low):
    y, _ = _run(x, w_high, w_low, act_scale_high, act_scale_low)
    return y


# revision 17
# speedup vs baseline: 1.0939x; 1.0233x over previous
"""DRQConv2d (dual-region quantized conv) Trainium2 kernel — v2.

Reference semantics:
  mask  = upsample8(avgpool8(x) >= 0.05)             per (b, c)
  xh    = where(mask, x, 1e-5);  xl = where(mask, 1e-5, x)
  qh    = clip(round(xh/sh), 0, 255) * sh            (uint8 fake-quant)
  ql    = clip(round(xl/sl), 0, 15) * sl             (uint4 fake-quant)
  qwh   = per-oc quant of w_high to +-127,  qwl = per-oc quant of w_low to +-7
  y     = conv3x3(qh, qwh) + conv3x3(ql, qwl)        (pad 1)

Key ideas vs the v1 baseline (151us):
  * Low conv runs in fp8 e4m3 with MatmulPerfMode.DoubleRow: quantized low
    activations (ints 0..15) and weights (ints +-7) are exactly representable
    in e4m3, and DoubleRow packs TWO 3x3 taps (2x128 contraction rows) into
    one PE instruction -> 9 taps in 5 matmuls instead of 9.  The per-oc scale
    ratio sv_h/sv_l is folded into the HIGH (bf16) weights instead of the low
    ones so the low weights stay exact integers; both convs share PSUM banks
    and one final evacuation scale by sv_l (adds ~0.1% error, gate is 2%).
  * The region mask is applied as a CLAMP BOUND instead of a multiply:
    v = x/s + MAGIC (ACT, scale+round fused); u = min(v, MAGIC + qmax*mask)
    (GpSimd, mask expanded only to [P, 7*56] block-row resolution and
    broadcast-viewed); q = max(u, MAGIC) - MAGIC (DVE tensor_scalar, 2x mode).
    Masked-out pixels clamp to exactly MAGIC -> quantize to 0, which matches
    the reference (1e-5 rounds to 0).  This removes the full-res mask
    expansion and the 1x-mode scalar_tensor_tensor of v1.
  * PSUM is laid out as two bank-aligned supertiles (chunks at 512-elem
    stride) so each image needs only 2 strided ACT evacuations + 1 DMA out.

Sharding: data-parallel over batch. 32 images -> 4 per core on 8 cores,
weights replicated; outputs concatenated on host. No collectives.
"""

import numpy as np

P = 128            # channels (both in and out) == partitions
B_TOTAL = 32
N_CORES = 8
BPC = B_TOTAL // N_CORES   # images per core
H = W = 56
HP = WP = H + 2    # zero-padded layout
NPIX = H * W       # 3136
NPAD = HP * WP     # 3364
NTAPS = 9
ROWS = 8
NCHUNK = H // ROWS                    # 7
NFREE = ROWS * W                      # 448 columns per matmul
BANK = 512                            # PSUM bank stride in f32 elements
MAGIC = float(np.float32(1.5 * 2 ** 23))   # fp32 round-to-nearest magic
POOL_K = 8
THRESH = 0.05


def build_program(nc, tc, aps, inv_sh, inv_sl, c_svh, c_svl, bpc=BPC):
    import bass_rust as _br
    import concourse.mybir as mybir
    from concourse.alu_op_type import AluOpType as op
    from concourse.masks import make_identity

    f32 = mybir.dt.float32
    bf16 = mybir.dt.bfloat16
    fp8 = mybir.dt.float8e4
    X = mybir.AxisListType.X
    DR = mybir.MatmulPerfMode.DoubleRow
    IDENT = mybir.ActivationFunctionType.Identity

    x_d, wh_d, wl_d, y_d = aps["x"], aps["w_high"], aps["w_low"], aps["y"]
    sum_thresh = float(np.float32(THRESH) * POOL_K * POOL_K)  # exact pow2 scale

    with (
        tc.tile_pool(name="consts", bufs=1) as consts,
        tc.tile_pool(name="wtmp", bufs=1) as wtmp,
        tc.tile_pool(name="psum", bufs=1, space="PSUM") as psum_pool,
        tc.tile_pool(name="acts", bufs=2) as acts,
        tc.tile_pool(name="qtiles", bufs=3) as qtiles,
        tc.tile_pool(name="outs", bufs=2) as outs_pool,
    ):
        identity = consts.tile([P, P], f32)
        make_identity(nc, identity[:])
        magic_ap = consts.tile([P, 1], f32, tag="magic", name="magic")
        nc.vector.memset(magic_ap[:], MAGIC)
        nmagic_ap = consts.tile([P, 1], f32, tag="nmagic", name="nmagic")
        nc.vector.memset(nmagic_ap[:], -MAGIC)
        # touch the ACT table immediately so ACT_TABLE_LOAD is off the
        # critical path of the first quant pass
        warm_act = consts.tile([P, 1], f32, tag="warm_act", name="warm_act")
        nc.scalar.activation(warm_act[:], magic_ap[:], IDENT)

        # PSUM: chunks 0-3 in psA banks 0-3, chunks 4-6 in psB banks 0-2;
        # psB bank 3 (the 8th PSUM bank) doubles as transpose/warmup scratch
        # (only touched before the first conv matmul).
        psA = psum_pool.tile([P, 4 * BANK], f32, tag="psA")
        psB = psum_pool.tile([P, 4 * BANK], f32, tag="psB")
        tp = psB[:, 3 * BANK:4 * BANK]

        # ---------------- DMA-in: x first (longest critical path) ----------
        xts = {}

        def issue_x(b):
            if b < bpc and b not in xts:
                xts[b] = acts.tile([P, NPIX], f32, tag="xt", name=f"xt{b}")
                nc.sync.dma_start(out=xts[b][:], in_=x_d[b])

        wnat_h = wtmp.tile([P, P * NTAPS], f32, tag="wnat_h")
        wnat_l = wtmp.tile([P, P * NTAPS], f32, tag="wnat_l")
        nc.sync.dma_start(out=wnat_h[:], in_=wh_d)
        nc.sync.dma_start(out=wnat_l[:], in_=wl_d)
        issue_x(0)

        # ---------------- per-image prep: mask -> bounds -> quantize --------
        qs = {}

        def prep(b):
            issue_x(b + 1)
            xt = xts[b]
            # w-blocksum written transposed so the h-blocksum reduce is a
            # single contiguous-group reduce
            r1 = acts.tile([P, H * NCHUNK], f32, tag="r1")
            nc.vector.reduce_sum(
                r1[:].rearrange("p (w h) -> p h w", w=NCHUNK),
                xt[:].rearrange("p (r c) -> p r c", c=POOL_K), axis=X)
            r2 = acts.tile([P, NCHUNK * NCHUNK], f32, tag="r2")
            nc.vector.reduce_sum(
                r2[:], r1[:].rearrange("p (g c) -> p g c", c=POOL_K), axis=X)
            mt = acts.tile([P, NCHUNK * NCHUNK], f32, tag="mt")
            nc.vector.tensor_scalar(mt[:], r2[:], sum_thresh, None, op0=op.is_ge)
            m = acts.tile([P, NCHUNK * NCHUNK], f32, tag="m")
            nc.vector.tensor_copy(
                out=m[:], in_=mt[:].rearrange("p (w h) -> p h w", w=NCHUNK))
            # clamp bounds in the MAGIC domain, at block resolution:
            # bound_h = MAGIC + 255*m ; bound_l = MAGIC + 15*(1-m)
            bh_blk = acts.tile([P, NCHUNK * NCHUNK], f32, tag="bh_blk")
            nc.vector.tensor_scalar(
                bh_blk[:], m[:], 255.0, MAGIC, op0=op.mult, op1=op.add)
            bl_blk = acts.tile([P, NCHUNK * NCHUNK], f32, tag="bl_blk")
            nc.vector.tensor_scalar(
                bl_blk[:], m[:], -15.0, MAGIC + 15.0, op0=op.mult, op1=op.add)
            bh = acts.tile([P, NCHUNK * W], f32, tag="bh")
            nc.vector.tensor_copy(
                out=bh[:].rearrange("p (g c) -> p g c", c=POOL_K),
                in_=bh_blk[:].unsqueeze(2).broadcast_to(
                    (P, NCHUNK * NCHUNK, POOL_K)))
            bl = acts.tile([P, NCHUNK * W], f32, tag="bl")
            nc.vector.tensor_copy(
                out=bl[:].rearrange("p (g c) -> p g c", c=POOL_K),
                in_=bl_blk[:].unsqueeze(2).broadcast_to(
                    (P, NCHUNK * NCHUNK, POOL_K)))

            def quant(conv, inv_s, bnd, qdt):
                # v = x/s + MAGIC (ACT); u = min(v, bound) (DVE, applies both
                # the qmax clip and the region mask); q = max(u,MAGIC)-MAGIC
                # cast to bf16/fp8 (DVE tensor_scalar, 2x mode)
                v = acts.tile([P, NPIX], f32, tag=f"v{conv}", name=f"v{conv}")
                nc.scalar.activation(
                    v[:], xt[:], IDENT, bias=magic_ap[:, 0:1], scale=inv_s)
                v4 = v[:].rearrange("p (hb r c) -> p hb r c", hb=NCHUNK, r=ROWS)
                bnd4 = (bnd[:].rearrange("p (hb c) -> p hb c", hb=NCHUNK)
                        .unsqueeze(2).broadcast_to((P, NCHUNK, ROWS, W)))
                nc.vector.tensor_tensor(v4, v4, bnd4, op=op.min)
                q = qtiles.tile([P, NPAD], qdt, tag=f"q{conv}", name=f"q{conv}")
                q3 = q[:].rearrange("p (r c) -> p r c", r=HP)
                nc.gpsimd.memset(q3[:, 0:HP:HP - 1, :], 0.0)
                nc.gpsimd.memset(q3[:, 1:HP - 1, 0:WP:WP - 1], 0.0)
                nc.vector.tensor_scalar(
                    q3[:, 1:H + 1, 1:W + 1],
                    v[:].rearrange("p (r c) -> p r c", r=H),
                    MAGIC, MAGIC, op0=op.max, op1=op.subtract)
                return q

            qs[b] = (quant("h", inv_sh, bh, bf16),
                     quant("l", inv_sl, bl, fp8))

        # ---------------- weight prep ----------------
        def pp(tag):
            return consts.tile([P, 1], f32, tag=tag, name=tag)

        am_h, am_l = pp("am_h"), pp("am_l")
        nc.vector.tensor_reduce(
            am_h[:], wnat_h[:], axis=X, op=op.max, apply_absolute_value=True)
        nc.vector.tensor_reduce(
            am_l[:], wnat_l[:], axis=X, op=op.max, apply_absolute_value=True)
        sv_l, sv_h = pp("sv_l"), pp("sv_h")
        nc.vector.tensor_scalar_mul(sv_l[:], am_l[:], c_svl)
        nc.vector.tensor_scalar_mul(sv_h[:], am_h[:], c_svh)
        rcp_svl, ratio = pp("rcp_svl"), pp("ratio")
        nc.vector.reciprocal(rcp_svl[:], sv_l[:])
        nc.vector.tensor_tensor(ratio[:], sv_h[:], rcp_svl[:], op=op.mult)
        rcp_h, rs_h = pp("rcp_h"), pp("rs_h")
        nc.vector.reciprocal(rcp_h[:], am_h[:])
        nc.vector.tensor_scalar_mul(rs_h[:], rcp_h[:], 127.0)
        rcp_l, rs_l = pp("rcp_l"), pp("rs_l")
        nc.vector.reciprocal(rcp_l[:], am_l[:])
        nc.vector.tensor_scalar_mul(rs_l[:], rcp_l[:], 7.0)

        # integer-quantize in natural [oc, ic*9] layout: the magic round
        # runs on the (otherwise idle) ACT engine -- a1 = w*rs + MAGIC,
        # a2 = a1 - MAGIC (Sterbenz-exact) -- and DVE only clamps/scales.
        nc.scalar.activation(
            wnat_h[:], wnat_h[:], IDENT, bias=magic_ap[:, 0:1],
            scale=rs_h[:, 0:1])
        nc.scalar.activation(
            wnat_h[:], wnat_h[:], IDENT, bias=nmagic_ap[:, 0:1], scale=1.0)
        # fold sv_h/sv_l into the high weights so both convs share one
        # output scale (sv_l); low weights stay exact fp8 integers
        nc.vector.tensor_scalar(
            wnat_h[:], wnat_h[:], 127.0, -127.0, op0=op.min, op1=op.max)
        nc.vector.tensor_scalar_mul(wnat_h[:], wnat_h[:], ratio[:, 0:1])
        nc.scalar.activation(
            wnat_l[:], wnat_l[:], IDENT, bias=magic_ap[:, 0:1],
            scale=rs_l[:, 0:1])
        nc.scalar.activation(
            wnat_l[:], wnat_l[:], IDENT, bias=nmagic_ap[:, 0:1], scale=1.0)
        nc.vector.tensor_scalar(
            wnat_l[:], wnat_l[:], 7.0, -7.0, op0=op.min, op1=op.max)

        # early PE activity starts the HAM clock ramp while DMAs land
        for i in range(10):
            nc.tensor.matmul(
                tp[:, 0:P], identity[:], identity[:],
                start=(i == 0), stop=(i == 9))

        # transpose each tap [oc, ic] -> [ic, oc]; cast to bf16 / fp8e4.
        # Transpose groups ping-pong between two PSUM scratch regions (psA is
        # still unused) so group N+1's transposes overlap group N's ACT copy.
        qwt_h = consts.tile([P, NTAPS * P], bf16, tag="qwt_h")
        qwt_l = consts.tile([P, NTAPS * P], fp8, tag="qwt_l")
        tp2 = psA[:, 0:BANK]
        gi = 0
        for wq, qwt in ((wnat_h, qwt_h), (wnat_l, qwt_l)):
            wv = wq[:].rearrange("p (i t) -> p t i", t=NTAPS)
            for base in range(0, NTAPS, 4):
                n = min(4, NTAPS - base)
                tp_r = tp2 if gi % 2 else tp
                gi += 1
                for j in range(n):
                    nc.tensor.transpose(
                        tp_r[:, j * P:(j + 1) * P], wv[:, base + j, :],
                        identity[:])
                # PSUM->SBUF evacuation on ACT (idle at startup); keeping it
                # off the DVE queue unblocks the transpose->conv chain
                nc.scalar.copy(
                    out=qwt[:, base * P:(base + n) * P], in_=tp_r[:, :n * P])

        # second warm-up burst right before the convs so the PE clock is at
        # full speed when the first real matmul issues
        for i in range(18):
            nc.tensor.matmul(
                tp[:, 0:P], identity[:], identity[:],
                start=(i == 0), stop=(i == 17))

        prep(0)

        # ---------------- convs ----------------
        def seg_of(c):
            return (psA[:, c * BANK:c * BANK + NFREE] if c < 4
                    else psB[:, (c - 4) * BANK:(c - 4) * BANK + NFREE])

        def conv(b):
            # high phase first (needs only qh), then the fp8 low phase --
            # gives prep(b) maximal slack to finish ql while PE runs high.
            qh, ql = qs.pop(b)
            acc = outs_pool.tile([P, NPIX], f32, tag="acc")
            qh3 = qh[:].rearrange("p (r c) -> p r c", r=HP)
            ql3 = ql[:].rearrange("p (r c) -> p r c", r=HP)
            qlf = ql[:]
            for c in range(NCHUNK):
                r0 = c * ROWS
                seg = seg_of(c)
                for tap in range(NTAPS):
                    kh, kw = divmod(tap, 3)
                    nc.tensor.matmul(
                        seg, qwt_h[:, tap * P:(tap + 1) * P],
                        qh3[:, r0 + kh:r0 + kh + ROWS, kw:kw + W],
                        start=(tap == 0), stop=False)
            for c in range(NCHUNK):
                r0 = c * ROWS
                seg = seg_of(c)
                for pr in range(4):
                    t0 = 2 * pr
                    kh0, kw0 = divmod(t0, 3)
                    kh1, kw1 = divmod(t0 + 1, 3)
                    o0 = (r0 + kh0) * HP + kw0
                    ds = (r0 + kh1) * HP + kw1 - o0
                    # overlapping strided view [P, 2(pair), 8(rows), 56(cols)]
                    rv = (qlf[:, 0:2 * ROWS * W]
                          .rearrange("p (a b c) -> p a b c", a=2, b=ROWS)
                          .copy())
                    rv.ap = _br.VecI64Pair(
                        [[NPAD, P], [ds, 2], [HP, ROWS], [1, W]])
                    rv.offset = qlf.offset + o0
                    lhsT = (qwt_l[:, t0 * P:(t0 + 2) * P]
                            .rearrange("p (two m) -> p two m", two=2))
                    nc.tensor.matmul(
                        seg, lhsT, rv, start=False, stop=False, perf_mode=DR)
                nc.tensor.matmul(
                    seg, qwt_l[:, 8 * P:9 * P],
                    ql3[:, r0 + 2:r0 + 2 + ROWS, 2:2 + W],
                    start=False, stop=True)
                if c == 3:
                    nc.scalar.mul(
                        acc[:, 0:4 * NFREE].rearrange("p (b k) -> p b k", b=4),
                        psA[:].rearrange("p (b k) -> p b k", b=4)[:, :, 0:NFREE],
                        sv_l[:, 0:1])
                    nc.sync.dma_start(
                        out=y_d[b][:, 0:4 * NFREE], in_=acc[:, 0:4 * NFREE])
                if c == 6:
                    nc.scalar.mul(
                        acc[:, 4 * NFREE:7 * NFREE]
                        .rearrange("p (b k) -> p b k", b=3),
                        psB[:, 0:3 * BANK]
                        .rearrange("p (b k) -> p b k", b=3)[:, :, 0:NFREE],
                        sv_l[:, 0:1])
                    nc.sync.dma_start(
                        out=y_d[b][:, 4 * NFREE:7 * NFREE],
                        in_=acc[:, 4 * NFREE:7 * NFREE])

        for b in range(bpc):
            if b + 1 < bpc:
                prep(b + 1)
            conv(b)


def make_bass(inv_sh, inv_sl, c_svh, c_svl, bpc=BPC):
    import concourse.bacc as bacc
    import concourse.mybir as mybir
    from concourse.tile import TileContext

    f32 = mybir.dt.float32
    nc = bacc.Bacc("TRN2", debug=False)
    x = nc.dram_tensor("x", [bpc, P, NPIX], f32, kind="ExternalInput")
    wh = nc.dram_tensor("w_high", [P, P * NTAPS], f32, kind="ExternalInput")
    wl = nc.dram_tensor("w_low", [P, P * NTAPS], f32, kind="ExternalInput")
    y = nc.dram_tensor("y", [bpc, P, NPIX], f32, kind="ExternalOutput")
    aps = {"x": x.ap(), "w_high": wh.ap(), "w_low": wl.ap(), "y": y.ap()}
    with TileContext(nc) as tc:
        build_program(nc, tc, aps, inv_sh, inv_sl, c_svh, c_svl, bpc=bpc)
    nc.compile()
    return nc


def _scale_consts(act_scale_high, act_scale_low):
    sh = float(np.float32(act_scale_high))
    sl = float(np.float32(act_scale_low))
    inv_sh = float(np.float32(1.0 / np.float64(sh)))
    inv_sl = float(np.float32(1.0 / np.float64(sl)))
    c_svh = float(np.float32(np.float64(sh) / 127.0))
    c_svl = float(np.float32(np.float64(sl) / 7.0))
    return inv_sh, inv_sl, c_svh, c_svl


def _run(x, w_high, w_low, act_scale_high, act_scale_low, trace=False, **kw):
    from concourse import bass_utils

    x = np.ascontiguousarray(np.asarray(x, dtype=np.float32))
    w_high = np.ascontiguousarray(np.asarray(w_high, dtype=np.float32))
    w_low = np.ascontiguousarray(np.asarray(w_low, dtype=np.float32))

    inv_sh, inv_sl, c_svh, c_svl = _scale_consts(act_scale_high, act_scale_low)
    nc = make_bass(inv_sh, inv_sl, c_svh, c_svl)

    wh_flat = w_high.reshape(P, P * NTAPS)
    wl_flat = w_low.reshape(P, P * NTAPS)
    in_maps = []
    for core in range(N_CORES):
        xs = x[core * BPC:(core + 1) * BPC].reshape(BPC, P, NPIX)
        in_maps.append(
            {
                "x": np.ascontiguousarray(xs),
                "w_high": wh_flat,
                "w_low": wl_flat,
            }
        )
    res = bass_utils.run_bass_kernel_spmd(
        nc, in_maps, core_ids=list(range(N_CORES)), trace=trace, **kw
    )
    y = np.concatenate([r["y"].reshape(BPC, P, H, W) for r in res.results], axis=0)
    return y, res


def kernel(x, w_high, w_low, act_scale_high, act_scale_low):
    y, _ = _run(x, w_high, w_low, act_scale_high, act_scale_low)
    return y
